# revision 58
# baseline (speedup 1.0000x reference)
"""Trainium2 Bass kernel for nn_Aggre_user (GNN message-passing aggregation).

Reference computation (per batch node, over its variable-length edge list):
    f      = relu(ln1(cat(user_emb, rating_emb)))            per edge
    h      = relu(att2(relu(att1(cat(f, item_emb[node])))))  per edge
    a      = att3(h)                                         per edge logit
    mu     = segment_softmax(a)
    z      = relu(ln2(segment_sum(f * mu)))                  per node
    out    = relu(ln3(cat(item_emb[node], z)))               per node

Sharding: nodes (B=8192) split contiguously across 8 cores (1024 each);
segment_ids are sorted, so each node's edges land wholly on one core.
No collectives needed.

Device-side strategy (per core), v2:
  - user table host-converted to bf16 [N, 128] (feats in cols 0:64); edges
    gathered with the ANT `dma_gather` ucode in TRANSPOSE mode straight into
    feature-major SBUF [128, n_idx] -- no on-chip transpose/convert of user
    data at all. Calls cover up to 56 tiles (7168 idxs) to amortize the
    ~1us/call SWDGE ucode overhead on the gpsimd engine.
  - dma_gather indices are int16, so user ids are range-sharded into
    SH=25000-id windows with window-local indices; edges of each node-group
    (W=64 nodes) are partitioned into one padded cell per shard.
  - 128-edge tiles processed in quads (8 tiles); tile pairs (t, t+4) share a
    128-partition "pack" column block: top half = feats of tile t, bottom =
    tile t+4, built directly by two 64-contraction matmuls from the gather
    buffer (plain 512-col slices; no data movement).
  - block-diagonal weights let one 512-col stream compute both pack halves
    for att1/att2; per-quad matmuls are merged into single instructions.
  - rating table folded into ln1 via host-built 5-hot and M_r = rating_table
    @ ln1_w[:,64:].T streamed as extra contraction rows.
  - att1's per-node item term via c1 = item @ att1_w[:,64:].T contracted
    against a segment one-hot built on-chip (seg-local ids vs iota); pairs
    whose halves live in different node-groups get per-half matmuls.
  - att3 logits computed EDGE-major by using h2 pack blocks as the matmul
    stationary operand (rhs = packed w3): [128, 2] out per pair; exp runs on
    the tiny [128, 8] tile.
  - segment softmax: no max-subtraction needed (logits O(1)); no explicit mu:
    z = (sum_e ex*f) / max(sum_e ex, 1e-9); both sums via one matmul with an
    ex-scaled one-hot and a ones column (per shard-cell partials, summed).
  - item embeddings use transpose-mode dma_gather from a host-padded bf16
    table [Nrows, 128] with an all-zero row per SH-id shard (off-shard
    indices masked to the zero row and the four passes summed).
"""

import math
import numpy as np
import ml_dtypes

import concourse.bass as bass
import concourse.mybir as mybir
import concourse.tile as tile
from concourse import bacc
from concourse.bass_utils import run_bass_kernel_spmd

BF16 = ml_dtypes.bfloat16
AF = mybir.ActivationFunctionType
N_CORES = 8
W = 64        # nodes per segment group
SH = 25000    # ids per gather shard
NSH = 4       # shards
MAX_CALL_TILES = 7    # tiles per user-gather call (896 idxs: the HW SWDGE
                      # ring fits <=~992 descriptors per call)
QSB_BUFS = 6          # quad-pipeline SBUF ring depth


def pad_table(t, zero_rows=True):
    """[N, 64] f32 -> bf16 [N(+zrows), 128] with feats in cols 0:64.

    zero_rows: insert an all-zero row after each SH-row window (for masked
    multi-pass gathers). Returns (table, pos) where pos[i] is the padded row
    of original row i.
    """
    t = np.asarray(t, np.float32)
    N = t.shape[0]
    if not zero_rows:
        out = np.zeros((N, 128), np.float32)
        out[:, :64] = t
        return out.astype(BF16), np.arange(N)
    nsh = (N + SH - 1) // SH
    out = np.zeros((N + nsh, 128), np.float32)
    pos = np.arange(N) + np.arange(N) // SH
    out[pos, :64] = t
    return out.astype(BF16), pos


def wrap16(idx_i16):
    """flat int16 index list (len % 16 == 0) -> [128, len//16] wrapped+replicated."""
    a = idx_i16.reshape(-1, 16).T  # [16, S]
    return np.tile(a, (8, 1)).copy()


def pair_of_tile(t):
    """tile index -> (pair index, half). Pairs are (t, t+4) within a quad."""
    q, t8 = divmod(t, 8)
    return (4 * q + (t8 % 4), t8 // 4)


# ----------------------------------------------------------------------------
# host-side preprocessing: shard + pad + relayout (pure index manipulation)
# ----------------------------------------------------------------------------

def host_prep(inputs, n_cores=N_CORES):
    user_idx = np.asarray(inputs["user_idx"]).astype(np.int64)
    rating_idx = np.asarray(inputs["rating_idx"]).astype(np.int64)
    item_idx = np.asarray(inputs["item_idx"]).astype(np.int64)
    seg = np.asarray(inputs["segment_ids"]).astype(np.int64)

    B = item_idx.shape[0]
    B_loc = B // n_cores
    assert B_loc % 128 == 0
    NG = B_loc // W
    n_groups = B // W

    utab, _ = pad_table(inputs["user_table"], zero_rows=False)
    itab, ipos = pad_table(inputs["item_table"], zero_rows=True)
    N_u = np.asarray(inputs["user_table"]).shape[0]
    N_i = np.asarray(inputs["item_table"]).shape[0]
    n_ush = (N_u + SH - 1) // SH
    n_ish = (N_i + SH - 1) // SH
    u_shard = user_idx // SH        # shard per edge

    bounds = np.searchsorted(seg, np.arange(n_groups + 1) * W)

    # per-(group, shard) cell counts -> per-shard cell capacity C_q
    cellcnt = np.zeros((n_groups, NSH), np.int64)
    for g in range(n_groups):
        lo, hi = bounds[g], bounds[g + 1]
        for q in range(NSH):
            cellcnt[g, q] = int((u_shard[lo:hi] == q).sum())
    align = 128
    zrow_i = [min(N_i - SH * q, SH) for q in range(n_ish)]
    Cq = [0 if q >= n_ush else
          int(math.ceil(max(1, int(cellcnt[:, q].max())) / align) * align)
          for q in range(NSH)]
    E_grp = sum(Cq)
    T = NG * E_grp // 128
    E_pad = NG * E_grp
    blk_tiles = [NG * c // 128 for c in Cq]   # tiles per shard block
    assert all(bt % 8 == 0 for bt in blk_tiles)
    group_of_tile = []
    for q in range(NSH):
        for g in range(NG):
            group_of_tile += [g] * (Cq[q] // 128)
    shard_of_tile = []
    for q in range(NSH):
        shard_of_tile += [q] * blk_tiles[q]

    # user-gather call chunks: within each shard block, runs of <=MAX tiles
    calls = []          # list of (t0, ntiles)
    t0 = 0
    for q in range(NSH):
        rem = blk_tiles[q]
        while rem > 0:
            nt = min(MAX_CALL_TILES, rem)
            calls.append((t0, nt))
            t0 += nt
            rem -= nt
    assert t0 == T
    call_of_tile = np.zeros(T, np.int64)
    for ci, (ct0, nt) in enumerate(calls):
        call_of_tile[ct0:ct0 + nt] = ci

    per_core = []
    blk0 = np.concatenate([[0], np.cumsum(blk_tiles)]) * 128  # slot offsets
    for k in range(n_cores):
        ugl = np.zeros(E_pad, np.int64)      # shard-local padded positions
        ridx = np.full(E_pad, -1, np.int64)
        sloc = np.full(E_pad, -1.0, np.float64)
        for gl in range(NG):
            g = NG * k + gl
            lo, hi = bounds[g], bounds[g + 1]
            esl = np.arange(lo, hi)
            shards_here = u_shard[lo:hi]
            for q in range(NSH):
                if Cq[q] == 0:
                    assert not (shards_here == q).any()
                    continue
                mine = esl[shards_here == q]
                c = len(mine)
                s = int(blk0[q]) + gl * Cq[q]
                assert c <= Cq[q]
                ugl[s:s + c] = user_idx[mine] - SH * q
                ugl[s + c:s + Cq[q]] = 0
                ridx[s:s + c] = rating_idx[mine]
                sloc[s:s + c] = seg[mine] - W * g
        assert (ugl >= 0).all() and (ugl < SH).all()
        uw = wrap16(ugl.astype(np.int16))    # [128, E_pad//16]
        # segl column order: within each quad, col 8q+2c+two holds tile
        # 8q+4*two+c, so the device-side [p, c, two] view has plain
        # decreasing strides (2, 1) -- HW DVE APs need monotonic strides.
        tile_perm = np.arange(T)
        for qd in range(T // 8):
            for kk in range(8):
                tile_perm[8 * qd + kk] = 8 * qd + 4 * (kk % 2) + (kk // 2)
        segl = sloc.reshape(T, 128)[tile_perm].T.astype(BF16).copy()
        segr = sloc.reshape(1, E_pad).astype(BF16).copy()
        rt = ridx.reshape(T, 128)
        P = T // 2
        oh5 = np.zeros((10, P, 128), np.float32)
        for t in range(T):
            pi, half = pair_of_tile(t)
            for r in range(5):
                oh5[5 * half + r, pi] = (rt[t] == r)
        oh5p = oh5.reshape(10, P * 128).astype(BF16)
        # item gather: n_ish passes in node order, off-shard -> shard zero row
        nodes = slice(B_loc * k, B_loc * (k + 1))
        it_loc = ipos[item_idx[nodes]]
        it_shard = item_idx[nodes] // SH
        iw = []
        for q in range(n_ish):
            loc = np.where(it_shard == q, it_loc - (SH + 1) * q, zrow_i[q])
            iw.append(wrap16(loc.astype(np.int16)))
        per_core.append(dict(
            uw=uw, segl=segl, segr=segr, oh5p=oh5p,
            iw=np.concatenate(iw, axis=1),
        ))

    # weights (tiny; fold rating table into ln1 on host)
    f32 = np.float32
    ln1_w = np.asarray(inputs["ln1_w"], f32)
    att1_w = np.asarray(inputs["att1_w"], f32)
    att2_w = np.asarray(inputs["att2_w"], f32)
    att3_w = np.asarray(inputs["att3_w"], f32)
    ln2_w = np.asarray(inputs["ln2_w"], f32)
    ln3_w = np.asarray(inputs["ln3_w"], f32)
    rating_table = np.asarray(inputs["rating_table"], f32)

    def bd(a):
        K, M = a.shape
        o = np.zeros((2 * K, 2 * M), f32)
        o[:K, :M] = a
        o[K:, M:] = a
        return o.astype(BF16)

    MrT = rating_table @ ln1_w[:, 64:].T
    w3 = att3_w[0]
    w3p = np.zeros((128, 2), f32)
    w3p[:64, 0] = w3
    w3p[64:, 1] = w3

    shared = dict(
        w1u=ln1_w[:, :64].T.astype(BF16), bd_mr=bd(MrT),
        bd_a1f=bd(att1_w[:, :64].T),
        a1it=att1_w[:, 64:].T.astype(BF16),
        bd_a2=bd(att2_w.T), w3p=w3p.astype(BF16),
        w2t=ln2_w.T.astype(BF16),
        w3it=ln3_w[:, :64].T.astype(BF16), w3zt=ln3_w[:, 64:].T.astype(BF16),
        b1p=np.tile(np.asarray(inputs["ln1_b"], f32), 2)[:, None],
        ba1p=np.tile(np.asarray(inputs["att1_b"], f32), 2)[:, None],
        ba2p=np.tile(np.asarray(inputs["att2_b"], f32), 2)[:, None],
        b2=np.asarray(inputs["ln2_b"], f32)[:, None],
        b3f=np.asarray(inputs["ln3_b"], f32)[:, None],
        iota64=np.tile(np.arange(W, dtype=f32), (128, 1)).astype(BF16),
        iota2=(np.arange(128) % W).astype(f32)[:, None],
        id_bf=np.eye(128, dtype=f32).astype(BF16),
        id_f32=np.eye(128, dtype=f32),
        utab=utab, itab=itab,
    )
    meta = dict(B=B, B_loc=B_loc, NG=NG, T=T, E_grp=E_grp, Cq=tuple(Cq),
                blk_tiles=tuple(blk_tiles), group_of_tile=tuple(group_of_tile),
                shard_of_tile=tuple(shard_of_tile),
                calls=tuple(calls), call_of_tile=tuple(call_of_tile),
                zrow_i=tuple(zrow_i),
                n_cores=n_cores, UR=utab.shape[0], IR=itab.shape[0],
                n_ush=n_ush, n_ish=n_ish)
    return per_core, shared, meta


# ----------------------------------------------------------------------------
# bass program builder
# ----------------------------------------------------------------------------

def build_nc_real(meta):
    NG, T = meta["NG"], meta["T"]
    B_loc = meta["B_loc"]
    Cq, blk_tiles = meta["Cq"], meta["blk_tiles"]
    got = meta["group_of_tile"]
    sot = meta["shard_of_tile"]
    calls = meta["calls"]
    cot = meta["call_of_tile"]

    nc = bacc.Bacc("TRN2", target_bir_lowering=False, debug=False,
                   enable_asserts=False, num_devices=meta["n_cores"])
    f32, bf16, i16 = mybir.dt.float32, mybir.dt.bfloat16, mybir.dt.int16

    def din(name, shape, dtype):
        return nc.dram_tensor(name, shape, dtype, kind="ExternalInput").ap()

    utab = din("utab", [meta["UR"], 128], bf16)
    itab = din("itab", [meta["IR"], 128], bf16)
    uw = din("uw", [128, T * 8], i16)
    segl = din("segl", [128, T], bf16)
    segr = din("segr", [1, T * 128], bf16)
    iota2 = din("iota2", [128, 1], f32)
    oh5p = din("oh5p", [10, 64 * T], bf16)
    iw = din("iw", [128, meta["n_ish"] * B_loc // 16], i16)
    iota64 = din("iota64", [128, W], bf16)
    id_bf = din("id_bf", [128, 128], bf16)
    id_f32 = din("id_f32", [128, 128], f32)
    w1u = din("w1u", [64, 64], bf16)
    bd_mr = din("bd_mr", [10, 128], bf16)
    bd_a1f = din("bd_a1f", [128, 128], bf16)
    a1it = din("a1it", [64, 64], bf16)
    bd_a2 = din("bd_a2", [128, 128], bf16)
    w3p = din("w3p", [128, 2], bf16)
    w2t = din("w2t", [64, 64], bf16)
    w3it = din("w3it", [64, 64], bf16)
    w3zt = din("w3zt", [64, 64], bf16)
    b1p = din("b1p", [128, 1], f32)
    ba1p = din("ba1p", [128, 1], f32)
    ba2p = din("ba2p", [128, 1], f32)
    b2 = din("b2", [64, 1], f32)
    b3f = din("b3f", [64, 1], f32)
    out = nc.dram_tensor("out", [B_loc, 64], f32, kind="ExternalOutput").ap()

    with tile.TileContext(nc) as tc:
        with (
            tc.tile_pool(name="const", bufs=1) as cpool,
            tc.tile_pool(name="core", bufs=1) as corep,
            tc.tile_pool(name="ug", bufs=8) as ugp,
            tc.tile_pool(name="qsb", bufs=QSB_BUFS) as qsb,
            tc.tile_pool(name="post", bufs=2) as postp,
            tc.tile_pool(name="pmm", bufs=3, space="PSUM") as pmm,
            tc.tile_pool(name="ptr", bufs=1, space="PSUM") as ptr,
            tc.tile_pool(name="pa", bufs=1, space="PSUM") as pa,
            tc.tile_pool(name="pg", bufs=2, space="PSUM") as pg,
            tc.tile_pool(name="pgp", bufs=1, space="PSUM") as pgp,
        ):
            def load(pool, ap, tag):
                t = pool.tile(list(ap.shape), ap.dtype, tag=tag, name=tag)
                nc.sync.dma_start(out=t[:], in_=ap)
                return t

            c_id_bf = load(cpool, id_bf, "id_bf")
            c_id_f32 = load(cpool, id_f32, "id_f32")
            c_iota = load(cpool, iota64, "iota")
            c_iota2 = load(cpool, iota2, "iota2")
            c_w1u = load(cpool, w1u, "w1u")
            c_bd_mr = load(cpool, bd_mr, "bd_mr")
            c_bd_a1f = load(cpool, bd_a1f, "bd_a1f")
            c_a1it = load(cpool, a1it, "a1it")
            c_bd_a2 = load(cpool, bd_a2, "bd_a2")
            c_w3p = load(cpool, w3p, "w3p")
            c_w2t = load(cpool, w2t, "w2t")
            c_w3it = load(cpool, w3it, "w3it")
            c_w3zt = load(cpool, w3zt, "w3zt")
            c_b1p = load(cpool, b1p, "b1p")
            c_ba1p = load(cpool, ba1p, "ba1p")
            c_ba2p = load(cpool, ba2p, "ba2p")
            c_b2 = load(cpool, b2, "b2")
            c_b3f = load(cpool, b3f, "b3f")
            c_segl = load(corep, segl, "segl")
            c_uw = load(corep, uw, "uw")
            c_oh5p = load(corep, oh5p, "oh5p")
            c_iw = load(corep, iw, "iw")

            for _rep in range(meta.get("repeat", 1)):
                # ---- item embeddings: zero-masked gathers per shard, summed ----
                S_it = B_loc // 16
                NCH = max(1, B_loc // 512)           # 512-idx chunks per pass
                ch = B_loc // NCH
                it_parts = []
                for q in range(meta["n_ish"]):
                    gq = corep.tile([128, 1, B_loc], bf16, tag=f"itg{q}",
                                    name=f"itg{q}")
                    base = (SH + 1) * q
                    rows = min(SH + 1, meta["IR"] - base)
                    for c in range(NCH):
                        nc.gpsimd.dma_gather(
                            out_ap=gq[:, :, c * ch:(c + 1) * ch],
                            in_ap=itab[base:base + rows, :],
                            idxs_ap=c_iw[:, q * S_it + c * ch // 16:
                                         q * S_it + (c + 1) * ch // 16],
                            num_idxs=ch, num_idxs_reg=ch,
                            elem_size=128, transpose=True)
                    it_parts.append(gq)
                itemT = corep.tile([128, B_loc], bf16, tag="itemT", name="itemT")
                nc.vector.tensor_tensor(out=itemT[:], in0=it_parts[0][:, 0, :],
                                        in1=it_parts[1][:, 0, :],
                                        op=mybir.AluOpType.add)
                for q in range(2, len(it_parts)):
                    nc.vector.tensor_tensor(out=itemT[:], in0=itemT[:],
                                            in1=it_parts[q][:, 0, :],
                                            op=mybir.AluOpType.add)

                def item_fm_slice(g):
                    return itemT[0:64, W * g:W * g + W]

                # ---- per-group c1 block-diag lhsT ----
                bd_c1 = corep.tile([128, NG, 128], bf16, tag="bd_c1", name="bd_c1")
                nc.gpsimd.memset(bd_c1[:], 0)
                for g in range(NG):
                    src = item_fm_slice(g)
                    ps = pgp.tile([128, 128], f32, tag="gp", name=f"c1ps{g}")
                    nc.tensor.matmul(ps[0:64, 0:64], lhsT=src, rhs=c_a1it[:],
                                     start=True, stop=True, skip_group_check=True)
                    nc.tensor.matmul(ps[64:128, 64:128], lhsT=src, rhs=c_a1it[:],
                                     start=True, stop=True, skip_group_check=True)
                    nc.vector.tensor_copy(out=bd_c1[0:64, g, 0:64],
                                          in_=ps[0:64, 0:64])
                    nc.vector.tensor_copy(out=bd_c1[64:128, g, 64:128],
                                          in_=ps[64:128, 64:128])

                # per-group accumulated G (f32, SBUF)
                G_all = corep.tile([65, NG, W], f32, tag="G_all", name="G_all")

                # persistent fT ring ([f*ex | ex], col 64 = ex written per use)
                ft_bufs = []
                for i in range(QSB_BUFS):
                    t = corep.tile([128, 8, 68], bf16, tag=f"ftb{i}",
                                   name=f"ftb{i}")
                    nc.vector.memset(t[:, :, 64:65], 1.0)
                    ft_bufs.append(t)

                def group_post(g):
                    G_sb = G_all[:, g, :]
                    Gt = pgp.tile([64, 65], f32, tag="gp", name=f"Gt{g}")
                    nc.tensor.transpose(out=Gt[:], in_=G_sb,
                                        identity=c_id_f32[0:65, 0:65])
                    den = postp.tile([64, 1], f32, tag="den", name=f"den{g}")
                    nc.vector.tensor_scalar_max(out=den[:], in0=Gt[:, 64:65],
                                                scalar1=1e-9)
                    rec = postp.tile([64, 1], f32, tag="rec", name=f"rec{g}")
                    nc.vector.reciprocal(out=rec[:], in_=den[:])
                    z_nm = postp.tile([64, W], bf16, tag="z_nm", name=f"znm{g}")
                    nc.vector.tensor_scalar_mul(out=z_nm[:], in0=Gt[:, 0:64],
                                                scalar1=rec[:, 0:1])
                    zf_ps = pgp.tile([64, 64], bf16, tag="gp", name=f"zf{g}")
                    nc.tensor.transpose(out=zf_ps[:], in_=z_nm[:],
                                        identity=c_id_bf[0:64, 0:64])
                    z_fm = postp.tile([64, 64], bf16, tag="z_fm", name=f"zfm{g}")
                    nc.vector.tensor_copy(out=z_fm[:], in_=zf_ps[:])
                    z2_ps = pgp.tile([64, 64], f32, tag="gp", name=f"z2ps{g}")
                    nc.tensor.matmul(z2_ps[:], lhsT=c_w2t[:], rhs=z_fm[:],
                                     start=True, stop=True, skip_group_check=True)
                    z2 = postp.tile([64, 64], bf16, tag="z2", name=f"z2{g}")
                    nc.scalar.activation(out=z2[:], in_=z2_ps[:], func=AF.Relu,
                                         bias=c_b2[:])
                    o_ps = pgp.tile([64, 64], f32, tag="gp", name=f"ops{g}")
                    nc.tensor.matmul(o_ps[:], lhsT=c_w3it[:], rhs=item_fm_slice(g),
                                     start=True, stop=False, skip_group_check=True)
                    nc.tensor.matmul(o_ps[:], lhsT=c_w3zt[:], rhs=z2[:],
                                     start=False, stop=True, skip_group_check=True)
                    o_fm = postp.tile([64, 64], f32, tag="o_fm", name=f"ofm{g}")
                    nc.scalar.activation(out=o_fm[:], in_=o_ps[:], func=AF.Relu,
                                         bias=c_b3f[:])
                    otr = pgp.tile([64, 64], f32, tag="gp", name=f"otr{g}")
                    nc.tensor.transpose(out=otr[:], in_=o_fm[:],
                                        identity=c_id_f32[0:64, 0:64])
                    o_sb = postp.tile([64, 64], f32, tag="o_sb", name=f"osb{g}")
                    nc.vector.tensor_copy(out=o_sb[:], in_=otr[:])
                    nc.sync.dma_start(out=out[W * g:W * g + W, :], in_=o_sb[:])

                # ---- main edge pipeline over gather calls / quads ----
                blk_first = [sum(blk_tiles[:q]) for q in range(NSH)]
                last_q = max(q for q in range(NSH) if blk_tiles[q] > 0)
                first_q = min(q for q in range(NSH) if blk_tiles[q] > 0)
                G_cell = {}
                u_call = {}

                def ensure_call(ci):
                    if ci in u_call or ci >= len(calls):
                        return
                    ct0, nt = calls[ci]
                    q_shard = sot[ct0]
                    base = SH * q_shard
                    rows = min(SH, meta["UR"] - base)
                    u_g = ugp.tile([128, 1, nt * 128], bf16, tag="u_g",
                                   name=f"ug{ci}")
                    nc.gpsimd.dma_gather(
                        out_ap=u_g[:], in_ap=utab[base:base + rows, :],
                        idxs_ap=c_uw[:, ct0 * 8:(ct0 + nt) * 8],
                        num_idxs=nt * 128, num_idxs_reg=nt * 128,
                        elem_size=128, transpose=True)
                    u_call[ci] = u_g

                def g_stage(qt0, fT, ohx):
                    """Segment-sum cell partials for one quad. Deferred one
                    quad behind the front stage so the PE stream never stalls
                    on the att3 -> exp -> ohx roundtrip."""
                    for t8 in range(8):
                        t = qt0 + t8
                        g = got[t]
                        q = sot[t]
                        ct = Cq[q] // 128
                        j = (t - blk_first[q]) % ct
                        # fT index of tile t8: pairs are (p, p+4) -> (2p, 2p+1)
                        fi = 2 * (t8 % 4) + (t8 // 4)
                        key = (g, q)
                        if j == 0:
                            G_cell[key] = pg.tile([65, W], f32, tag="G",
                                                  name=f"G{g}_{q}")
                        G_ps = G_cell[key]
                        nc.tensor.matmul(
                            G_ps[:], lhsT=fT[:, fi, 0:65],
                            rhs=ohx[:, t8 % 4, t8 // 4, :],
                            start=(j == 0), stop=(j == ct - 1),
                            skip_group_check=True)
                        if j == ct - 1:
                            if q == first_q:
                                nc.vector.tensor_copy(out=G_all[:, g, :],
                                                      in_=G_ps[:])
                            else:
                                nc.vector.tensor_tensor(
                                    out=G_all[:, g, :], in0=G_all[:, g, :],
                                    in1=G_ps[:], op=mybir.AluOpType.add)
                            del G_cell[key]
                            if q == last_q:
                                group_post(g)

                def half_quad_rhs(t0h):
                    """[(call_buf, col_off, ncols), ...] covering 4 tiles
                    (512 slots) starting at tile t0h; may span two calls."""
                    parts = []
                    need = 512
                    t = t0h
                    while need > 0:
                        ci = cot[t]
                        ct0, nt = calls[ci]
                        off = (t - ct0) * 128
                        n = min(need, nt * 128 - off)
                        parts.append((ci, off, n))
                        need -= n
                        t += n // 128
                    return parts

                def stage_a(qt0):
                    """ln1 matmuls + f relu. Runs one quad ahead of stage_b so
                    the Act engine always has a ready f_ps."""
                    for la in range(cot[qt0], min(cot[qt0] + 4, len(calls))):
                        ensure_call(la)
                    f_ps = pmm.tile([128, 512], f32, tag="mm")
                    for half, prange in ((0, slice(0, 64)), (1, slice(64, 128))):
                        cs0 = 0
                        # PSUM group "start" zeroes the whole 2KB bank row:
                        # only the first piece per half may set it.
                        for ci, off, n in half_quad_rhs(qt0 + 4 * half):
                            nc.tensor.matmul(
                                f_ps[prange, cs0:cs0 + n], lhsT=c_w1u[:],
                                rhs=u_call[ci][0:64, 0, off:off + n],
                                start=(cs0 == 0), stop=False,
                                skip_group_check=True)
                            cs0 += n
                    nc.tensor.matmul(f_ps[:, :], lhsT=c_bd_mr[:],
                                     rhs=c_oh5p[:, 64 * qt0:64 * qt0 + 512],
                                     start=False, stop=True, skip_group_check=True)
                    f_sb = qsb.tile([128, 512], bf16, tag="f")
                    nc.scalar.activation(out=f_sb[:], in_=f_ps[:], func=AF.Relu,
                                         bias=c_b1p[:])
                    return f_sb

                def stage_b1(qt0, f_sb):
                    # --- segment one-hots (edge-major), viewed [128, 4, 2, W]
                    # (pair index outer, half inner: slice [:, p] is contiguous)
                    oh2 = qsb.tile([128, 4, 2, W], bf16, tag="oh2")
                    nc.vector.tensor_tensor(
                        out=oh2[:],
                        in0=c_segl[:, qt0:qt0 + 8]
                            .rearrange("p (c two o) -> p c two o", two=2, o=1)
                            .to_broadcast([128, 4, 2, W]),
                        in1=c_iota[:].rearrange("p (o1 o2 n) -> p o1 o2 n",
                                                o1=1, o2=1)
                            .to_broadcast([128, 4, 2, W]),
                        op=mybir.AluOpType.is_equal,
                    )

                    # transposed one-hot per pair, built directly: broadcast
                    # the slot-ordered seg row across partitions by stride-0
                    # DMA (idle SP engine), then one is_equal against a
                    # per-partition iota -- no PE transposes, no PSUM trip.
                    segb = qsb.tile([128, 4, 128], bf16, tag="segb")
                    for half in (0, 1):
                        nc.sync.dma_start(
                            out=segb[64 * half:64 * half + 64, :, :],
                            in_=segr[0:1, (qt0 + 4 * half) * 128:
                                     (qt0 + 4 * half + 4) * 128]
                                .rearrange("o (c j) -> o c j", j=128)
                                .to_broadcast([64, 4, 128]))
                    oh1 = qsb.tile([128, 512], bf16, tag="oh1")
                    nc.vector.tensor_scalar(
                        out=oh1[:], in0=segb[:].rearrange("p c j -> p (c j)"),
                        scalar1=c_iota2[:, 0:1], scalar2=0.0,
                        op0=mybir.AluOpType.is_equal,
                        op1=mybir.AluOpType.bypass)

                    # --- att1 (merged f-term + per-pair item terms) ---
                    h1_ps = pmm.tile([128, 512], f32, tag="mm")
                    nc.tensor.matmul(h1_ps[:], lhsT=c_bd_a1f[:], rhs=f_sb[:],
                                     start=True, stop=False, skip_group_check=True)
                    p = 0
                    while p < 4:
                        gt, gb = got[qt0 + p], got[qt0 + 4 + p]
                        if gt == gb:
                            p2 = p + 1
                            while (p2 < 4 and got[qt0 + p2] == gt
                                   and got[qt0 + 4 + p2] == gt):
                                p2 += 1
                            cs = slice(128 * p, 128 * p2)
                            nc.tensor.matmul(h1_ps[:, cs], lhsT=bd_c1[:, gt, :],
                                             rhs=oh1[:, cs], start=False,
                                             stop=True, skip_group_check=True)
                            p = p2
                        else:
                            cs = slice(128 * p, 128 * (p + 1))
                            nc.tensor.matmul(h1_ps[0:64, cs],
                                             lhsT=bd_c1[0:64, gt, 0:64],
                                             rhs=oh1[0:64, cs], start=False,
                                             stop=True, skip_group_check=True)
                            nc.tensor.matmul(h1_ps[64:128, cs],
                                             lhsT=bd_c1[64:128, gb, 64:128],
                                             rhs=oh1[64:128, cs], start=False,
                                             stop=True, skip_group_check=True)
                            p += 1
                    return h1_ps, oh2

                def stage_b2(qt0, f_sb, h1_ps, oh2):
                    h1 = qsb.tile([128, 512], bf16, tag="h1")
                    nc.scalar.activation(out=h1[:], in_=h1_ps[:], func=AF.Relu,
                                         bias=c_ba1p[:])
                    # --- att2 ---
                    h2_ps = pmm.tile([128, 512], f32, tag="mm")
                    nc.tensor.matmul(h2_ps[:], lhsT=c_bd_a2[:], rhs=h1[:],
                                     start=True, stop=True, skip_group_check=True)
                    h2 = qsb.tile([128, 512], bf16, tag="h2")
                    nc.scalar.activation(out=h2[:], in_=h2_ps[:],
                                         func=AF.Relu, bias=c_ba2p[:])

                    # --- att3 logits EDGE-major (h2 pack as stationary) + exp ---
                    aT_ps = pa.tile([128, 8], f32, tag="a")
                    for p in range(4):
                        nc.tensor.matmul(aT_ps[:, 2 * p:2 * p + 2],
                                         lhsT=h2[:, 128 * p:128 * (p + 1)],
                                         rhs=c_w3p[:], start=True, stop=True,
                                         skip_group_check=True)
                    ex_em = qsb.tile([128, 8], bf16, tag="ex_em")
                    nc.scalar.activation(out=ex_em[:], in_=aT_ps[:], func=AF.Exp)

                    # --- f to edge-major, ex-scaled: fT = [f*ex | ex] ---
                    # Folding the softmax numerator into fT lets G consume the
                    # UNSCALED one-hot oh2 directly (no separate ohx op):
                    # G = sum_e [f_e*ex_e; ex_e] (x) oh[e, :].
                    fT_ps = ptr.tile([128, 512], bf16, tag="tr")
                    for p in range(4):
                        nc.tensor.transpose(
                            out=fT_ps[:, 128 * p:128 * (p + 1)],
                            in_=f_sb[:, 128 * p:128 * (p + 1)],
                            identity=c_id_bf[:],
                        )
                    fT = ft_bufs[(qt0 // 8) % QSB_BUFS]
                    nc.vector.tensor_copy(
                        out=fT[:, :, 0:64],
                        in_=fT_ps[:].rearrange("p (c d) -> p c d", d=64),
                    )
                    # --- ohx = oh2 * ex ---
                    ohx = qsb.tile([128, 4, 2, W], bf16, tag="ohx")
                    nc.vector.tensor_tensor(
                        out=ohx[:], in0=oh2[:],
                        in1=ex_em[:].rearrange("p (c two o) -> p c two o",
                                               two=2, o=1)
                            .to_broadcast([128, 4, 2, W]),
                        op=mybir.AluOpType.mult,
                    )

                    return fT, ohx

                # --- 4-stage software pipeline:
                #     A(k) | B1(k-1) | B2(k-2) | C(k-3)
                nq = T // 8
                fsb_q = {}
                b1_q = {}
                b2_q = {}
                for k in range(nq + 3):
                    if k < nq:
                        fsb_q[k] = stage_a(k * 8)
                    if 0 <= k - 1 < nq:
                        b1_q[k - 1] = stage_b1((k - 1) * 8, fsb_q[k - 1])
                    if 0 <= k - 2 < nq:
                        h1_ps, oh2 = b1_q.pop(k - 2)
                        b2_q[k - 2] = stage_b2((k - 2) * 8, fsb_q.pop(k - 2),
                                               h1_ps, oh2)
                    if 0 <= k - 3 < nq:
                        g_stage((k - 3) * 8, *b2_q.pop(k - 3))
    nc.finalize()
    return nc


# ----------------------------------------------------------------------------
# public entry point
# ----------------------------------------------------------------------------

_CACHE = {}


def _get_nc(meta):
    key = (meta["T"], meta["NG"], meta["B_loc"], meta["n_cores"], meta["Cq"],
           meta["UR"], meta["IR"], meta["calls"], meta["group_of_tile"])
    if key not in _CACHE:
        _CACHE[key] = build_nc_real(meta)
    return _CACHE[key]


def make_in_maps(per_core, shared, meta):
    in_maps = []
    for k in range(meta["n_cores"]):
        m = dict(shared)
        m.update(per_core[k])
        in_maps.append(m)
    return in_maps


def kernel(**inputs):
    per_core, shared, meta = host_prep(inputs, N_CORES)
    nc = _get_nc(meta)
    in_maps = make_in_maps(per_core, shared, meta)
    res = run_bass_kernel_spmd(nc, in_maps, core_ids=list(range(N_CORES)))
    outs = [res.results[k]["out"] for k in range(N_CORES)]
    return np.concatenate(outs, axis=0).astype(np.float32)


# revision 64
# speedup vs baseline: 1.2041x; 1.2041x over previous
"""Trainium2 Bass kernel for nn_Aggre_user (GNN message-passing aggregation).

Reference computation (per batch node, over its variable-length edge list):
    f      = relu(ln1(cat(user_emb, rating_emb)))            per edge
    h      = relu(att2(relu(att1(cat(f, item_emb[node])))))  per edge
    a      = att3(h)                                         per edge logit
    mu     = segment_softmax(a)
    z      = relu(ln2(segment_sum(f * mu)))                  per node
    out    = relu(ln3(cat(item_emb[node], z)))               per node

Sharding: nodes (B=8192) split contiguously across 8 cores (1024 each);
segment_ids are sorted, so each node's edges land wholly on one core.
No collectives needed.

Device-side strategy (per core), v2:
  - user table host-converted to bf16 [N, 128] (feats in cols 0:64); edges
    gathered with the ANT `dma_gather` ucode in TRANSPOSE mode straight into
    feature-major SBUF [128, n_idx] -- no on-chip transpose/convert of user
    data at all. Calls cover up to 56 tiles (7168 idxs) to amortize the
    ~1us/call SWDGE ucode overhead on the gpsimd engine.
  - dma_gather indices are int16, so user ids are range-sharded into
    SH=25000-id windows with window-local indices; edges of each node-group
    (W=64 nodes) are partitioned into one padded cell per shard.
  - 128-edge tiles processed in quads (8 tiles); tile pairs (t, t+4) share a
    128-partition "pack" column block: top half = feats of tile t, bottom =
    tile t+4, built directly by two 64-contraction matmuls from the gather
    buffer (plain 512-col slices; no data movement).
  - block-diagonal weights let one 512-col stream compute both pack halves
    for att1/att2; per-quad matmuls are merged into single instructions.
  - rating table folded into ln1 via host-built 5-hot and M_r = rating_table
    @ ln1_w[:,64:].T streamed as extra contraction rows.
  - att1's per-node item term via c1 = item @ att1_w[:,64:].T contracted
    against a segment one-hot built on-chip (seg-local ids vs iota); pairs
    whose halves live in different node-groups get per-half matmuls.
  - att3 logits computed EDGE-major by using h2 pack blocks as the matmul
    stationary operand (rhs = packed w3): [128, 2] out per pair; exp runs on
    the tiny [128, 8] tile.
  - segment softmax: no max-subtraction needed (logits O(1)); no explicit mu:
    z = (sum_e ex*f) / max(sum_e ex, 1e-9); both sums via one matmul with an
    ex-scaled one-hot and a ones column (per shard-cell partials, summed).
  - item embeddings use transpose-mode dma_gather from a host-padded bf16
    table [Nrows, 128] with an all-zero row per SH-id shard (off-shard
    indices masked to the zero row and the four passes summed).
"""

import math
import numpy as np
import ml_dtypes

import concourse.bass as bass
import concourse.mybir as mybir
import concourse.tile as tile
from concourse import bacc
from concourse.bass_utils import run_bass_kernel_spmd

BF16 = ml_dtypes.bfloat16
AF = mybir.ActivationFunctionType
N_CORES = 8
W = 64        # nodes per segment group
SH = 25000    # ids per gather shard
NSH = 4       # shards
MAX_CALL_TILES = 7    # tiles per user-gather call (896 idxs: the HW SWDGE
                      # ring fits <=~992 descriptors per call)
QSB_BUFS = 6          # quad-pipeline SBUF ring depth


def pad_table(t, zero_rows=True):
    """[N, 64] f32 -> bf16 [N(+zrows), 128] with feats in cols 0:64.

    zero_rows: insert an all-zero row after each SH-row window (for masked
    multi-pass gathers). Returns (table, pos) where pos[i] is the padded row
    of original row i.
    """
    t = np.asarray(t, np.float32)
    N = t.shape[0]
    if not zero_rows:
        out = np.zeros((N, 128), np.float32)
        out[:, :64] = t
        return out.astype(BF16), np.arange(N)
    nsh = (N + SH - 1) // SH
    out = np.zeros((N + nsh, 128), np.float32)
    pos = np.arange(N) + np.arange(N) // SH
    out[pos, :64] = t
    return out.astype(BF16), pos


def wrap16(idx_i16):
    """flat int16 index list (len % 16 == 0) -> [128, len//16] wrapped+replicated."""
    a = idx_i16.reshape(-1, 16).T  # [16, S]
    return np.tile(a, (8, 1)).copy()


def pair_of_tile(t):
    """tile index -> (pair index, half). Pairs are (t, t+4) within a quad."""
    q, t8 = divmod(t, 8)
    return (4 * q + (t8 % 4), t8 // 4)


# ----------------------------------------------------------------------------
# host-side preprocessing: shard + pad + relayout (pure index manipulation)
# ----------------------------------------------------------------------------

def host_prep(inputs, n_cores=N_CORES):
    user_idx = np.asarray(inputs["user_idx"]).astype(np.int64)
    rating_idx = np.asarray(inputs["rating_idx"]).astype(np.int64)
    item_idx = np.asarray(inputs["item_idx"]).astype(np.int64)
    seg = np.asarray(inputs["segment_ids"]).astype(np.int64)

    B = item_idx.shape[0]
    B_loc = B // n_cores
    assert B_loc % 128 == 0
    NG = B_loc // W
    n_groups = B // W

    utab, _ = pad_table(inputs["user_table"], zero_rows=False)
    itab, ipos = pad_table(inputs["item_table"], zero_rows=True)
    N_u = np.asarray(inputs["user_table"]).shape[0]
    N_i = np.asarray(inputs["item_table"]).shape[0]
    n_ush = (N_u + SH - 1) // SH
    n_ish = (N_i + SH - 1) // SH
    u_shard = user_idx // SH        # shard per edge

    bounds = np.searchsorted(seg, np.arange(n_groups + 1) * W)

    # per-(group, shard) cell counts -> per-shard cell capacity C_q
    cellcnt = np.zeros((n_groups, NSH), np.int64)
    for g in range(n_groups):
        lo, hi = bounds[g], bounds[g + 1]
        for q in range(NSH):
            cellcnt[g, q] = int((u_shard[lo:hi] == q).sum())
    align = 128
    zrow_i = [min(N_i - SH * q, SH) for q in range(n_ish)]
    Cq = [0 if q >= n_ush else
          int(math.ceil(max(1, int(cellcnt[:, q].max())) / align) * align)
          for q in range(NSH)]
    E_grp = sum(Cq)
    T = NG * E_grp // 128
    E_pad = NG * E_grp
    blk_tiles = [NG * c // 128 for c in Cq]   # tiles per shard block
    assert all(bt % 8 == 0 for bt in blk_tiles)
    group_of_tile = []
    for q in range(NSH):
        for g in range(NG):
            group_of_tile += [g] * (Cq[q] // 128)
    shard_of_tile = []
    for q in range(NSH):
        shard_of_tile += [q] * blk_tiles[q]

    # user-gather call chunks: within each shard block, runs of <=MAX tiles
    calls = []          # list of (t0, ntiles)
    t0 = 0
    for q in range(NSH):
        rem = blk_tiles[q]
        while rem > 0:
            nt = min(MAX_CALL_TILES, rem)
            calls.append((t0, nt))
            t0 += nt
            rem -= nt
    assert t0 == T
    call_of_tile = np.zeros(T, np.int64)
    for ci, (ct0, nt) in enumerate(calls):
        call_of_tile[ct0:ct0 + nt] = ci

    per_core = []
    blk0 = np.concatenate([[0], np.cumsum(blk_tiles)]) * 128  # slot offsets
    for k in range(n_cores):
        ugl = np.zeros(E_pad, np.int64)      # shard-local padded positions
        ridx = np.full(E_pad, -1, np.int64)
        sloc = np.full(E_pad, -1.0, np.float64)
        for gl in range(NG):
            g = NG * k + gl
            lo, hi = bounds[g], bounds[g + 1]
            esl = np.arange(lo, hi)
            shards_here = u_shard[lo:hi]
            for q in range(NSH):
                if Cq[q] == 0:
                    assert not (shards_here == q).any()
                    continue
                mine = esl[shards_here == q]
                c = len(mine)
                s = int(blk0[q]) + gl * Cq[q]
                assert c <= Cq[q]
                ugl[s:s + c] = user_idx[mine] - SH * q
                ugl[s + c:s + Cq[q]] = 0
                ridx[s:s + c] = rating_idx[mine]
                sloc[s:s + c] = seg[mine] - W * g
        assert (ugl >= 0).all() and (ugl < SH).all()
        uw = wrap16(ugl.astype(np.int16))    # [128, E_pad//16]
        # segl column order: within each quad, col 8q+2c+two holds tile
        # 8q+4*two+c, so the device-side [p, c, two] view has plain
        # decreasing strides (2, 1) -- HW DVE APs need monotonic strides.
        tile_perm = np.arange(T)
        for qd in range(T // 8):
            for kk in range(8):
                tile_perm[8 * qd + kk] = 8 * qd + 4 * (kk % 2) + (kk // 2)
        segl = sloc.reshape(T, 128)[tile_perm].T.astype(BF16).copy()
        segr = sloc.reshape(1, E_pad).astype(BF16).copy()
        rt = ridx.reshape(T, 128)
        P = T // 2
        oh5 = np.zeros((10, P, 128), np.float32)
        for t in range(T):
            pi, half = pair_of_tile(t)
            for r in range(5):
                oh5[5 * half + r, pi] = (rt[t] == r)
        oh5p = oh5.reshape(10, P * 128).astype(BF16)
        # item gather: n_ish passes in node order, off-shard -> shard zero row
        nodes = slice(B_loc * k, B_loc * (k + 1))
        it_loc = ipos[item_idx[nodes]]
        it_shard = item_idx[nodes] // SH
        iw = []
        for q in range(n_ish):
            loc = np.where(it_shard == q, it_loc - (SH + 1) * q, zrow_i[q])
            iw.append(wrap16(loc.astype(np.int16)))
        per_core.append(dict(
            uw=uw, segl=segl, segr=segr, oh5p=oh5p,
            iw=np.concatenate(iw, axis=1),
        ))

    # weights (tiny; fold rating table into ln1 on host)
    f32 = np.float32
    ln1_w = np.asarray(inputs["ln1_w"], f32)
    att1_w = np.asarray(inputs["att1_w"], f32)
    att2_w = np.asarray(inputs["att2_w"], f32)
    att3_w = np.asarray(inputs["att3_w"], f32)
    ln2_w = np.asarray(inputs["ln2_w"], f32)
    ln3_w = np.asarray(inputs["ln3_w"], f32)
    rating_table = np.asarray(inputs["rating_table"], f32)

    def bd(a):
        K, M = a.shape
        o = np.zeros((2 * K, 2 * M), f32)
        o[:K, :M] = a
        o[K:, M:] = a
        return o.astype(BF16)

    MrT = rating_table @ ln1_w[:, 64:].T
    w3 = att3_w[0]
    w3p = np.zeros((128, 2), f32)
    w3p[:64, 0] = w3
    w3p[64:, 1] = w3

    shared = dict(
        w1u=ln1_w[:, :64].T.astype(BF16), bd_mr=bd(MrT),
        bd_a1f=bd(att1_w[:, :64].T),
        a1it=att1_w[:, 64:].T.astype(BF16),
        bd_a2=bd(att2_w.T), w3p=w3p.astype(BF16),
        w2t=ln2_w.T.astype(BF16),
        w3it=ln3_w[:, :64].T.astype(BF16), w3zt=ln3_w[:, 64:].T.astype(BF16),
        b1p=np.tile(np.asarray(inputs["ln1_b"], f32), 2)[:, None],
        ba1p=np.tile(np.asarray(inputs["att1_b"], f32), 2)[:, None],
        ba2p=np.tile(np.asarray(inputs["att2_b"], f32), 2)[:, None],
        b2=np.asarray(inputs["ln2_b"], f32)[:, None],
        b3f=np.asarray(inputs["ln3_b"], f32)[:, None],
        iota64=np.tile(np.arange(W, dtype=f32), (128, 1)).astype(BF16),
        iota2=(np.arange(128) % W).astype(f32)[:, None],
        id_bf=np.eye(128, dtype=f32).astype(BF16),
        id_f32=np.eye(128, dtype=f32),
        utab=utab, itab=itab,
    )
    meta = dict(B=B, B_loc=B_loc, NG=NG, T=T, E_grp=E_grp, Cq=tuple(Cq),
                blk_tiles=tuple(blk_tiles), group_of_tile=tuple(group_of_tile),
                shard_of_tile=tuple(shard_of_tile),
                calls=tuple(calls), call_of_tile=tuple(call_of_tile),
                zrow_i=tuple(zrow_i),
                n_cores=n_cores, UR=utab.shape[0], IR=itab.shape[0],
                n_ush=n_ush, n_ish=n_ish)
    return per_core, shared, meta


# ----------------------------------------------------------------------------
# bass program builder
# ----------------------------------------------------------------------------

def build_nc_real(meta):
    NG, T = meta["NG"], meta["T"]
    B_loc = meta["B_loc"]
    Cq, blk_tiles = meta["Cq"], meta["blk_tiles"]
    got = meta["group_of_tile"]
    sot = meta["shard_of_tile"]
    calls = meta["calls"]
    cot = meta["call_of_tile"]

    nc = bacc.Bacc("TRN2", target_bir_lowering=False, debug=False,
                   enable_asserts=False, num_devices=meta["n_cores"])
    f32, bf16, i16 = mybir.dt.float32, mybir.dt.bfloat16, mybir.dt.int16

    def din(name, shape, dtype):
        return nc.dram_tensor(name, shape, dtype, kind="ExternalInput").ap()

    utab = din("utab", [meta["UR"], 128], bf16)
    itab = din("itab", [meta["IR"], 128], bf16)
    uw = din("uw", [128, T * 8], i16)
    segl = din("segl", [128, T], bf16)
    segr = din("segr", [1, T * 128], bf16)
    iota2 = din("iota2", [128, 1], f32)
    oh5p = din("oh5p", [10, 64 * T], bf16)
    iw = din("iw", [128, meta["n_ish"] * B_loc // 16], i16)
    iota64 = din("iota64", [128, W], bf16)
    id_bf = din("id_bf", [128, 128], bf16)
    id_f32 = din("id_f32", [128, 128], f32)
    w1u = din("w1u", [64, 64], bf16)
    bd_mr = din("bd_mr", [10, 128], bf16)
    bd_a1f = din("bd_a1f", [128, 128], bf16)
    a1it = din("a1it", [64, 64], bf16)
    bd_a2 = din("bd_a2", [128, 128], bf16)
    w3p = din("w3p", [128, 2], bf16)
    w2t = din("w2t", [64, 64], bf16)
    w3it = din("w3it", [64, 64], bf16)
    w3zt = din("w3zt", [64, 64], bf16)
    b1p = din("b1p", [128, 1], f32)
    ba1p = din("ba1p", [128, 1], f32)
    ba2p = din("ba2p", [128, 1], f32)
    b2 = din("b2", [64, 1], f32)
    b3f = din("b3f", [64, 1], f32)
    out = nc.dram_tensor("out", [B_loc, 64], f32, kind="ExternalOutput").ap()

    with tile.TileContext(nc) as tc:
        with (
            tc.tile_pool(name="const", bufs=1) as cpool,
            tc.tile_pool(name="core", bufs=1) as corep,
            tc.tile_pool(name="ug", bufs=8) as ugp,
            tc.tile_pool(name="qsb", bufs=QSB_BUFS) as qsb,
            tc.tile_pool(name="post", bufs=2) as postp,
            tc.tile_pool(name="pmm", bufs=3, space="PSUM") as pmm,
            tc.tile_pool(name="ptr", bufs=1, space="PSUM") as ptr,
            tc.tile_pool(name="pa", bufs=1, space="PSUM") as pa,
            tc.tile_pool(name="pg", bufs=1, space="PSUM") as pg,
            tc.tile_pool(name="pgp", bufs=2, space="PSUM") as pgp,
        ):
            def load(pool, ap, tag):
                t = pool.tile(list(ap.shape), ap.dtype, tag=tag, name=tag)
                nc.sync.dma_start(out=t[:], in_=ap)
                return t

            c_id_bf = load(cpool, id_bf, "id_bf")
            c_id_f32 = load(cpool, id_f32, "id_f32")
            c_iota = load(cpool, iota64, "iota")
            c_iota2 = load(cpool, iota2, "iota2")
            c_w1u = load(cpool, w1u, "w1u")
            c_bd_mr = load(cpool, bd_mr, "bd_mr")
            c_bd_a1f = load(cpool, bd_a1f, "bd_a1f")
            c_a1it = load(cpool, a1it, "a1it")
            c_bd_a2 = load(cpool, bd_a2, "bd_a2")
            c_w3p = load(cpool, w3p, "w3p")
            c_w2t = load(cpool, w2t, "w2t")
            c_w3it = load(cpool, w3it, "w3it")
            c_w3zt = load(cpool, w3zt, "w3zt")
            c_b1p = load(cpool, b1p, "b1p")
            c_ba1p = load(cpool, ba1p, "ba1p")
            c_ba2p = load(cpool, ba2p, "ba2p")
            c_b2 = load(cpool, b2, "b2")
            c_b3f = load(cpool, b3f, "b3f")
            c_segl = load(corep, segl, "segl")
            c_uw = load(corep, uw, "uw")
            c_oh5p = load(corep, oh5p, "oh5p")
            c_iw = load(corep, iw, "iw")

            for _rep in range(meta.get("repeat", 1)):
                # ---- item embeddings: zero-masked gathers per shard, summed ----
                S_it = B_loc // 16
                NCH = max(1, B_loc // 512)           # 512-idx chunks per pass
                ch = B_loc // NCH
                it_parts = []
                for q in range(meta["n_ish"]):
                    gq = corep.tile([128, 1, B_loc], bf16, tag=f"itg{q}",
                                    name=f"itg{q}")
                    base = (SH + 1) * q
                    rows = min(SH + 1, meta["IR"] - base)
                    for c in range(NCH):
                        nc.gpsimd.dma_gather(
                            out_ap=gq[:, :, c * ch:(c + 1) * ch],
                            in_ap=itab[base:base + rows, :],
                            idxs_ap=c_iw[:, q * S_it + c * ch // 16:
                                         q * S_it + (c + 1) * ch // 16],
                            num_idxs=ch, num_idxs_reg=ch,
                            elem_size=128, transpose=True)
                    it_parts.append(gq)
                itemT = corep.tile([128, B_loc], bf16, tag="itemT", name="itemT")
                nc.vector.tensor_tensor(out=itemT[:], in0=it_parts[0][:, 0, :],
                                        in1=it_parts[1][:, 0, :],
                                        op=mybir.AluOpType.add)
                for q in range(2, len(it_parts)):
                    nc.vector.tensor_tensor(out=itemT[:], in0=itemT[:],
                                            in1=it_parts[q][:, 0, :],
                                            op=mybir.AluOpType.add)

                def item_fm_slice(g):
                    return itemT[0:64, W * g:W * g + W]

                # ---- per-group c1 block-diag lhsT ----
                bd_c1 = corep.tile([128, NG, 128], bf16, tag="bd_c1", name="bd_c1")
                nc.gpsimd.memset(bd_c1[:], 0)
                for g in range(NG):
                    src = item_fm_slice(g)
                    ps = pgp.tile([128, 128], f32, tag="gp", name=f"c1ps{g}")
                    nc.tensor.matmul(ps[0:64, 0:64], lhsT=src, rhs=c_a1it[:],
                                     start=True, stop=True, skip_group_check=True)
                    nc.tensor.matmul(ps[64:128, 64:128], lhsT=src, rhs=c_a1it[:],
                                     start=True, stop=True, skip_group_check=True)
                    nc.vector.tensor_copy(out=bd_c1[0:64, g, 0:64],
                                          in_=ps[0:64, 0:64])
                    nc.vector.tensor_copy(out=bd_c1[64:128, g, 64:128],
                                          in_=ps[64:128, 64:128])

                # per-group accumulated G (f32, SBUF)
                G_all = corep.tile([65, NG, W], f32, tag="G_all", name="G_all")

                # persistent fT ring ([f*ex | ex], col 64 = ex written per use)
                ft_bufs = []
                for i in range(QSB_BUFS):
                    t = corep.tile([128, 8, 68], bf16, tag=f"ftb{i}",
                                   name=f"ftb{i}")
                    nc.vector.memset(t[:, :, 64:65], 1.0)
                    ft_bufs.append(t)

                def group_post(g0):
                    """Finalize TWO adjacent groups (g0, g0+1) in one batch:
                    every op doubles its partition- or column-width at the
                    same instruction cost, halving the serial post chains."""
                    G_sb = G_all[:, g0:g0 + 2, :].rearrange("p g w -> p (g w)")
                    Gt = pgp.tile([128, 65], f32, tag="gp", name=f"Gt{g0}")
                    nc.tensor.transpose(out=Gt[:], in_=G_sb,
                                        identity=c_id_f32[0:65, 0:65])
                    den = postp.tile([128, 1], f32, tag="den", name=f"den{g0}")
                    nc.vector.tensor_scalar_max(out=den[:], in0=Gt[:, 64:65],
                                                scalar1=1e-9)
                    rec = postp.tile([128, 1], f32, tag="rec", name=f"rec{g0}")
                    nc.vector.reciprocal(out=rec[:], in_=den[:])
                    z_nm = postp.tile([128, W], bf16, tag="z_nm", name=f"znm{g0}")
                    nc.vector.tensor_scalar_mul(out=z_nm[:], in0=Gt[:, 0:64],
                                                scalar1=rec[:, 0:1])
                    zf_ps = pgp.tile([64, 128], bf16, tag="gp", name=f"zf{g0}")
                    nc.tensor.transpose(out=zf_ps[:], in_=z_nm[:],
                                        identity=c_id_bf[:])
                    z_fm = postp.tile([64, 128], bf16, tag="z_fm", name=f"zfm{g0}")
                    nc.vector.tensor_copy(out=z_fm[:], in_=zf_ps[:])
                    z2_ps = pgp.tile([64, 128], f32, tag="gp", name=f"z2ps{g0}")
                    nc.tensor.matmul(z2_ps[:], lhsT=c_w2t[:], rhs=z_fm[:],
                                     start=True, stop=True, skip_group_check=True)
                    z2 = postp.tile([64, 128], bf16, tag="z2", name=f"z2{g0}")
                    nc.scalar.activation(out=z2[:], in_=z2_ps[:], func=AF.Relu,
                                         bias=c_b2[:])
                    o_ps = pgp.tile([64, 128], f32, tag="gp", name=f"ops{g0}")
                    nc.tensor.matmul(o_ps[:], lhsT=c_w3it[:],
                                     rhs=itemT[0:64, W * g0:W * g0 + 2 * W],
                                     start=True, stop=False, skip_group_check=True)
                    nc.tensor.matmul(o_ps[:], lhsT=c_w3zt[:], rhs=z2[:],
                                     start=False, stop=True, skip_group_check=True)
                    o_fm = postp.tile([64, 128], f32, tag="o_fm", name=f"ofm{g0}")
                    nc.scalar.activation(out=o_fm[:], in_=o_ps[:], func=AF.Relu,
                                         bias=c_b3f[:])
                    otr = pgp.tile([128, 64], f32, tag="gp", name=f"otr{g0}")
                    nc.tensor.transpose(out=otr[:], in_=o_fm[:],
                                        identity=c_id_f32[0:64, 0:64])
                    o_sb = postp.tile([128, 64], f32, tag="o_sb", name=f"osb{g0}")
                    nc.vector.tensor_copy(out=o_sb[:], in_=otr[:])
                    nc.sync.dma_start(out=out[W * g0:W * g0 + 2 * W, :],
                                      in_=o_sb[:])

                # ---- main edge pipeline over gather calls / quads ----
                blk_first = [sum(blk_tiles[:q]) for q in range(NSH)]
                last_q = max(q for q in range(NSH) if blk_tiles[q] > 0)
                first_q = min(q for q in range(NSH) if blk_tiles[q] > 0)
                G_cell = {}
                u_call = {}

                def ensure_call(ci):
                    if ci in u_call or ci >= len(calls):
                        return
                    ct0, nt = calls[ci]
                    q_shard = sot[ct0]
                    base = SH * q_shard
                    rows = min(SH, meta["UR"] - base)
                    u_g = ugp.tile([128, 1, nt * 128], bf16, tag="u_g",
                                   name=f"ug{ci}")
                    nc.gpsimd.dma_gather(
                        out_ap=u_g[:], in_ap=utab[base:base + rows, :],
                        idxs_ap=c_uw[:, ct0 * 8:(ct0 + nt) * 8],
                        num_idxs=nt * 128, num_idxs_reg=nt * 128,
                        elem_size=128, transpose=True)
                    u_call[ci] = u_g

                def g_stage(qt0, fT, ohx):
                    """Segment-sum cell partials for one quad. Deferred one
                    quad behind the front stage so the PE stream never stalls
                    on the att3 -> exp -> ohx roundtrip."""
                    for t8 in range(8):
                        t = qt0 + t8
                        g = got[t]
                        q = sot[t]
                        ct = Cq[q] // 128
                        j = (t - blk_first[q]) % ct
                        # fT index of tile t8: pairs are (p, p+4) -> (2p, 2p+1)
                        fi = 2 * (t8 % 4) + (t8 // 4)
                        key = (g, q)
                        if j == 0:
                            G_cell[key] = pg.tile([65, W], f32, tag="G",
                                                  name=f"G{g}_{q}")
                        G_ps = G_cell[key]
                        nc.tensor.matmul(
                            G_ps[:], lhsT=fT[:, fi, 0:65],
                            rhs=ohx[:, t8 % 4, t8 // 4, :],
                            start=(j == 0), stop=(j == ct - 1),
                            skip_group_check=True)
                        if j == ct - 1:
                            if q == first_q:
                                nc.vector.tensor_copy(out=G_all[:, g, :],
                                                      in_=G_ps[:])
                            else:
                                nc.vector.tensor_tensor(
                                    out=G_all[:, g, :], in0=G_all[:, g, :],
                                    in1=G_ps[:], op=mybir.AluOpType.add)
                            del G_cell[key]
                            if q == last_q and g % 2 == 1:
                                group_post(g - 1)

                def half_quad_rhs(t0h):
                    """[(call_buf, col_off, ncols), ...] covering 4 tiles
                    (512 slots) starting at tile t0h; may span two calls."""
                    parts = []
                    need = 512
                    t = t0h
                    while need > 0:
                        ci = cot[t]
                        ct0, nt = calls[ci]
                        off = (t - ct0) * 128
                        n = min(need, nt * 128 - off)
                        parts.append((ci, off, n))
                        need -= n
                        t += n // 128
                    return parts

                def stage_a(qt0):
                    """ln1 matmuls + f relu. Runs one quad ahead of stage_b so
                    the Act engine always has a ready f_ps."""
                    for la in range(cot[qt0], min(cot[qt0] + 4, len(calls))):
                        ensure_call(la)
                    f_ps = pmm.tile([128, 512], f32, tag="mm")
                    for half, prange in ((0, slice(0, 64)), (1, slice(64, 128))):
                        cs0 = 0
                        # PSUM group "start" zeroes the whole 2KB bank row:
                        # only the first piece per half may set it.
                        for ci, off, n in half_quad_rhs(qt0 + 4 * half):
                            nc.tensor.matmul(
                                f_ps[prange, cs0:cs0 + n], lhsT=c_w1u[:],
                                rhs=u_call[ci][0:64, 0, off:off + n],
                                start=(cs0 == 0), stop=False,
                                skip_group_check=True)
                            cs0 += n
                    nc.tensor.matmul(f_ps[:, :], lhsT=c_bd_mr[:],
                                     rhs=c_oh5p[:, 64 * qt0:64 * qt0 + 512],
                                     start=False, stop=True, skip_group_check=True)
                    f_sb = qsb.tile([128, 512], bf16, tag="f")
                    nc.scalar.activation(out=f_sb[:], in_=f_ps[:], func=AF.Relu,
                                         bias=c_b1p[:])
                    # prefetch the seg-row broadcast one stage early so B1's
                    # oh1 is_equal never waits on the DMA latency
                    segb = qsb.tile([128, 4, 128], bf16, tag="segb")
                    for half in (0, 1):
                        nc.sync.dma_start(
                            out=segb[64 * half:64 * half + 64, :, :],
                            in_=segr[0:1, (qt0 + 4 * half) * 128:
                                     (qt0 + 4 * half + 4) * 128]
                                .rearrange("o (c j) -> o c j", j=128)
                                .to_broadcast([64, 4, 128]))
                    return f_sb, segb

                def stage_b1(qt0, f_sb, segb):
                    # --- segment one-hots (edge-major), viewed [128, 4, 2, W]
                    # (pair index outer, half inner: slice [:, p] is contiguous)
                    oh2 = qsb.tile([128, 4, 2, W], bf16, tag="oh2")
                    nc.vector.tensor_tensor(
                        out=oh2[:],
                        in0=c_segl[:, qt0:qt0 + 8]
                            .rearrange("p (c two o) -> p c two o", two=2, o=1)
                            .to_broadcast([128, 4, 2, W]),
                        in1=c_iota[:].rearrange("p (o1 o2 n) -> p o1 o2 n",
                                                o1=1, o2=1)
                            .to_broadcast([128, 4, 2, W]),
                        op=mybir.AluOpType.is_equal,
                    )

                    # transposed one-hot per pair, built directly from the
                    # prefetched seg-row broadcast (see stage_a): one is_equal
                    # against a per-partition iota -- no PE transposes.
                    oh1 = qsb.tile([128, 512], bf16, tag="oh1")
                    nc.vector.tensor_scalar(
                        out=oh1[:], in0=segb[:].rearrange("p c j -> p (c j)"),
                        scalar1=c_iota2[:, 0:1], scalar2=0.0,
                        op0=mybir.AluOpType.is_equal,
                        op1=mybir.AluOpType.bypass)

                    # --- att1 (merged f-term + per-pair item terms) ---
                    h1_ps = pmm.tile([128, 512], f32, tag="mm")
                    nc.tensor.matmul(h1_ps[:], lhsT=c_bd_a1f[:], rhs=f_sb[:],
                                     start=True, stop=False, skip_group_check=True)
                    p = 0
                    while p < 4:
                        gt, gb = got[qt0 + p], got[qt0 + 4 + p]
                        if gt == gb:
                            p2 = p + 1
                            while (p2 < 4 and got[qt0 + p2] == gt
                                   and got[qt0 + 4 + p2] == gt):
                                p2 += 1
                            cs = slice(128 * p, 128 * p2)
                            nc.tensor.matmul(h1_ps[:, cs], lhsT=bd_c1[:, gt, :],
                                             rhs=oh1[:, cs], start=False,
                                             stop=True, skip_group_check=True)
                            p = p2
                        else:
                            cs = slice(128 * p, 128 * (p + 1))
                            nc.tensor.matmul(h1_ps[0:64, cs],
                                             lhsT=bd_c1[0:64, gt, 0:64],
                                             rhs=oh1[0:64, cs], start=False,
                                             stop=True, skip_group_check=True)
                            nc.tensor.matmul(h1_ps[64:128, cs],
                                             lhsT=bd_c1[64:128, gb, 64:128],
                                             rhs=oh1[64:128, cs], start=False,
                                             stop=True, skip_group_check=True)
                            p += 1
                    return h1_ps, oh2

                def stage_b2(qt0, f_sb, h1_ps, oh2):
                    h1 = qsb.tile([128, 512], bf16, tag="h1")
                    nc.scalar.activation(out=h1[:], in_=h1_ps[:], func=AF.Relu,
                                         bias=c_ba1p[:])
                    # --- att2 ---
                    h2_ps = pmm.tile([128, 512], f32, tag="mm")
                    nc.tensor.matmul(h2_ps[:], lhsT=c_bd_a2[:], rhs=h1[:],
                                     start=True, stop=True, skip_group_check=True)
                    h2 = qsb.tile([128, 512], bf16, tag="h2")
                    nc.scalar.activation(out=h2[:], in_=h2_ps[:],
                                         func=AF.Relu, bias=c_ba2p[:])

                    # --- att3 logits EDGE-major (h2 pack as stationary) + exp ---
                    aT_ps = pa.tile([128, 8], f32, tag="a")
                    for p in range(4):
                        nc.tensor.matmul(aT_ps[:, 2 * p:2 * p + 2],
                                         lhsT=h2[:, 128 * p:128 * (p + 1)],
                                         rhs=c_w3p[:], start=True, stop=True,
                                         skip_group_check=True)
                    ex_em = qsb.tile([128, 8], bf16, tag="ex_em")
                    nc.scalar.activation(out=ex_em[:], in_=aT_ps[:], func=AF.Exp)

                    # --- f to edge-major, ex-scaled: fT = [f*ex | ex] ---
                    # Folding the softmax numerator into fT lets G consume the
                    # UNSCALED one-hot oh2 directly (no separate ohx op):
                    # G = sum_e [f_e*ex_e; ex_e] (x) oh[e, :].
                    fT_ps = ptr.tile([128, 512], bf16, tag="tr")
                    for p in range(4):
                        nc.tensor.transpose(
                            out=fT_ps[:, 128 * p:128 * (p + 1)],
                            in_=f_sb[:, 128 * p:128 * (p + 1)],
                            identity=c_id_bf[:],
                        )
                    fT = ft_bufs[(qt0 // 8) % QSB_BUFS]
                    nc.vector.tensor_copy(
                        out=fT[:, :, 0:64],
                        in_=fT_ps[:].rearrange("p (c d) -> p c d", d=64),
                    )
                    # --- ohx = oh2 * ex ---
                    ohx = qsb.tile([128, 4, 2, W], bf16, tag="ohx")
                    nc.vector.tensor_tensor(
                        out=ohx[:], in0=oh2[:],
                        in1=ex_em[:].rearrange("p (c two o) -> p c two o",
                                               two=2, o=1)
                            .to_broadcast([128, 4, 2, W]),
                        op=mybir.AluOpType.mult,
                    )

                    return fT, ohx

                # --- 4-stage software pipeline:
                #     A(k) | B1(k-1) | B2(k-2) | C(k-3)
                nq = T // 8
                fsb_q = {}
                b1_q = {}
                b2_q = {}
                for k in range(nq + 3):
                    if k < nq:
                        fsb_q[k] = stage_a(k * 8)
                    if 0 <= k - 1 < nq:
                        f_sb, segb = fsb_q[k - 1]
                        b1_q[k - 1] = stage_b1((k - 1) * 8, f_sb, segb)
                    if 0 <= k - 2 < nq:
                        h1_ps, oh2 = b1_q.pop(k - 2)
                        b2_q[k - 2] = stage_b2((k - 2) * 8,
                                               fsb_q.pop(k - 2)[0],
                                               h1_ps, oh2)
                    if 0 <= k - 3 < nq:
                        g_stage((k - 3) * 8, *b2_q.pop(k - 3))
    nc.finalize()
    return nc


# ----------------------------------------------------------------------------
# public entry point
# ----------------------------------------------------------------------------

_CACHE = {}


def _get_nc(meta):
    key = (meta["T"], meta["NG"], meta["B_loc"], meta["n_cores"], meta["Cq"],
           meta["UR"], meta["IR"], meta["calls"], meta["group_of_tile"])
    if key not in _CACHE:
        _CACHE[key] = build_nc_real(meta)
    return _CACHE[key]


def make_in_maps(per_core, shared, meta):
    in_maps = []
    for k in range(meta["n_cores"]):
        m = dict(shared)
        m.update(per_core[k])
        in_maps.append(m)
    return in_maps


def kernel(**inputs):
    per_core, shared, meta = host_prep(inputs, N_CORES)
    nc = _get_nc(meta)
    in_maps = make_in_maps(per_core, shared, meta)
    res = run_bass_kernel_spmd(nc, in_maps, core_ids=list(range(N_CORES)))
    outs = [res.results[k]["out"] for k in range(N_CORES)]
    return np.concatenate(outs, axis=0).astype(np.float32)
